# revision 3
# baseline (speedup 1.0000x reference)
"""Trainium2 Bass kernel for CharPredictorMultirateFFN.

Model: emb = emb_table[tokens]; conv = relu(causal_conv1d(emb, K=16) + b);
logits = cat(emb, conv) @ lin_w.T + lin_b; out = softmax(logits).

Key algebraic restructure (tokens take only V=256 values):
  conv[s, h] = sum_k U[tok[s-15+k], k, h]   with U[v,k,h] = sum_e emb[v,e] conv_w[h,e,k]
so the conv becomes 16 shifted one-hot matmuls with contract dim 256 (half the
FLOPs of the direct E=512 conv) and the one-hot operand is exact in fp8.
The emb half of the final linear folds into P1 = emb_table @ lin_w[:, :E].T
(one-hot matmul, [256,256]), removing the embedding gather entirely.

fp8 DoubleRow: the PE's MatmulPerfMode.DoubleRow packs the 2 v-halves of the
256-wide one-hot contraction into ONE matmul (lhsT [128,2,128], rhs [128,2,512])
running at 2x the fp16 MAC rate. U is quantized to e4m3 host-side (the one-hot
is exact in fp8); measured end-to-end rel_l2 ~1.5e-2 vs the 2e-2 gate.

Sharding: data-parallel over batch — 4 sequences per core on 8 cores, all
tables replicated, no collectives.

biases are folded host-side: conv_b into U[:, K-1, :] (tap k=15 is always
valid for every output position), lin_b into P1 rows (shift-0 one-hot always
valid), so the device kernel has no bias adds.
"""

import numpy as np
import ml_dtypes

B, S, V, E, H, K = 32, 2048, 256, 512, 1024, 16
NCORES = 8
SEQ_PER_CORE = B // NCORES            # 4
PAD = K - 1                           # 15
SPAD = S + PAD                        # 2063
H8 = H // 128                         # 8
NTT = S // 512                        # 4 token-tiles of 512 per sequence
F16 = np.float16
F8 = ml_dtypes.float8_e4m3

TRACE = False          # set True (e.g. from test.py) to capture NTFF profile
LAST_RESULT = None     # BassKernelResults of the most recent run

_NC_CACHE = {}


def _build_nc(seq_per_core=SEQ_PER_CORE, ntt=NTT):
    """Build the Bass module (SPMD, identical program on every core)."""
    from contextlib import ExitStack
    import concourse.bacc as bacc
    import concourse.tile as tile
    import concourse.mybir as mybir

    f32 = mybir.dt.float32
    f16 = mybir.dt.float16
    f8 = mybir.dt.float8e4
    AF = mybir.ActivationFunctionType
    DR = mybir.MatmulPerfMode.DoubleRow
    toks = seq_per_core * ntt * 512

    nc = bacc.Bacc("TRN2", target_bir_lowering=False, debug=False,
                   num_devices=NCORES)

    # u layout: [part, g(h-half), k, vh, i(h-chunk), 128] so each conv group's
    # weights are contiguous per-k slabs that can land early via sliced DMAs.
    # The lhsT AP for DoubleRow is u[:, g, k, :, i, :] = [128, 2(vh), 128].
    oh_d = nc.dram_tensor("oh", [128, 2, seq_per_core, SPAD], f8,
                          kind="ExternalInput").ap()
    u_d = nc.dram_tensor("u", [128, 2, K, 2, 4, 128], f8,
                         kind="ExternalInput").ap()
    w2_d = nc.dram_tensor("w2", [128, H8, V], f16,
                          kind="ExternalInput").ap()
    # host-gathered P1[tok] rows (emb half of the linear; lin_b folded in):
    # [tile, p, m, v] = row tile*512 + m*128 + p, so each token-tile is one
    # contiguous [128, 4, V] DMA.
    pe_d = nc.dram_tensor("pe", [seq_per_core * ntt, 128, 4, V], f32,
                          kind="ExternalInput").ap()
    out_d = nc.dram_tensor("out", [toks, V], f32, kind="ExternalOutput").ap()

    with tile.TileContext(nc) as tc, ExitStack() as ctx:
        consts = ctx.enter_context(tc.tile_pool(name="consts", bufs=1))
        u_t = consts.tile([128, 2, K, 2, 4, 128], f8, name="u_t")
        oh_t = consts.tile([128, 2, seq_per_core, SPAD], f8, name="oh_t")
        w2_t = consts.tile([128, H8, V], f16, name="w2_t")
        # staggered loads ordered along the kernel's critical path: the
        # first conv group consumes u[:, 0, k] in k order on oh[b=0,
        # cols<528], so stream those slabs first in small chunks. The oh
        # head chunk goes out on the Activation HWDGE queue so its issue
        # doesn't serialize behind the u chunks on SP.
        nc.scalar.dma_start(oh_t[:, :, 0, 0:528], oh_d[:, :, 0, 0:528])
        for kq in range(K):
            eng = nc.sync if kq % 2 == 0 else nc.scalar
            eng.dma_start(u_t[:, 0, kq:kq + 1], u_d[:, 0, kq:kq + 1])
        KQ = 2
        for kq in range(0, K, KQ):
            nc.sync.dma_start(u_t[:, 1, kq:kq + KQ],
                              u_d[:, 1, kq:kq + KQ])
        nc.sync.dma_start(oh_t[:, :, 0, 528:SPAD], oh_d[:, :, 0, 528:SPAD])
        for b in range(1, seq_per_core):
            nc.sync.dma_start(oh_t[:, :, b, :], oh_d[:, :, b, :])
        nc.sync.dma_start(w2_t[:], w2_d[:])

        pe_pool = ctx.enter_context(tc.tile_pool(name="pep", bufs=3))
        r_pool = ctx.enter_context(tc.tile_pool(name="rp", bufs=3))
        cps = ctx.enter_context(tc.tile_pool(name="cps", bufs=6, space="PSUM"))

        # PE warm-up: the HAM clock gate holds the PE at 1.2 GHz until it
        # has been busy ~3.4us. Run throwaway matmuls while the input DMAs
        # are in flight so the real stream starts at 2.4 GHz with no cold
        # ramp. Operands are deliberately uninitialized (no memset: that
        # would wait on the Vector engine's preamble and delay the PE);
        # the results land in a PSUM bank that the first real accumulation
        # group resets via start=True.
        wlhs = consts.tile([128, 128], f16, name="wlhs")
        wrhs = consts.tile([128, 512], f16, name="wrhs")
        nc.gpsimd.memset(wlhs[:], 0)
        nc.gpsimd.memset(wrhs[:], 0)
        wp = cps.tile([128, 512], f32, name="warmps", tag="cp")
        for _ in range(10):
            nc.tensor.matmul(wp[:], wlhs[:], wrhs[:], start=True, stop=True)
        lps = ctx.enter_context(tc.tile_pool(name="lps", bufs=2, space="PSUM"))
        sm_pool = ctx.enter_context(tc.tile_pool(name="smp", bufs=4))
        out_pool = ctx.enter_context(tc.tile_pool(name="outp", bufs=4))

        def conv_group(b, tt, g, rt):
            """One conv h-half for 512 tokens -> relu -> rt[:, g*4:(g+1)*4]."""
            col0 = tt * 512
            ps = [cps.tile([128, 512], f32, name=f"cp{i}", tag="cp")
                  for i in range(4)]
            for k in range(K):
                rhs = oh_t[:, :, b, col0 + k: col0 + k + 512]
                for i in range(4):
                    nc.tensor.matmul(
                        ps[i][:],
                        u_t[:, g, k, :, i, :],
                        rhs,
                        start=(k == 0), stop=(k == K - 1),
                        perf_mode=DR)
            for i in range(4):
                nc.scalar.activation(rt[:, g * 4 + i, :], ps[i][:], AF.Relu)

        def stage3_m(b, tt, rt, pe_t, m):
            """One 128-token block: logits = R@W2T + P1 rows, softmax, DMA."""
            psl = lps.tile([128, V], f32, name="psl", tag="psl")
            for h8 in range(H8):
                nc.tensor.matmul(
                    psl[:], rt[:, h8, m * 128:(m + 1) * 128],
                    w2_t[:, h8, :],
                    start=(h8 == 0), stop=(h8 == H8 - 1))
            li = sm_pool.tile([128, V], f32, name="li", tag="li")
            nc.vector.tensor_add(li[:], psl[:], pe_t[:, m, :])
            et = sm_pool.tile([128, V], f32, name="et", tag="et")
            ssum = sm_pool.tile([128, 1], f32, name="ssum", tag="ssum")
            nc.scalar.activation(et[:], li[:], AF.Exp, accum_out=ssum[:])
            rec = sm_pool.tile([128, 1], f32, name="rec", tag="rec")
            nc.vector.reciprocal(rec[:], ssum[:])
            ot = out_pool.tile([128, V], f32, name="ot", tag="ot")
            nc.vector.tensor_scalar_mul(ot[:], et[:], rec[:])
            row0 = (b * ntt + tt) * 512 + m * 128
            nc.sync.dma_start(out_d[row0:row0 + 128, :], ot[:])

        # software pipeline at conv-group granularity: half of tile i-1's
        # stage3 runs on the PE between tile i's two conv groups, so each
        # group's relu has a ~1.7us window before its PSUM banks are reused
        # (no PE stall on the relu) and the final tile's tail is halved.
        tiles = [(b, tt) for b in range(seq_per_core) for tt in range(ntt)]
        prev = None
        for (b, tt) in tiles:
            pe_t = pe_pool.tile([128, 4, V], f32, name="pe_t", tag="pe")
            nc.sync.dma_start(pe_t[:], pe_d[b * ntt + tt])
            rt = r_pool.tile([128, H8, 512], f16, name="rt", tag="rt")
            conv_group(b, tt, 0, rt)
            if prev is not None:
                stage3_m(*prev, 0)
                stage3_m(*prev, 1)
            conv_group(b, tt, 1, rt)
            if prev is not None:
                stage3_m(*prev, 2)
                stage3_m(*prev, 3)
            prev = (b, tt, rt, pe_t)
        for m in range(4):
            stage3_m(*prev, m)

    nc.compile()
    return nc


def _get_nc():
    if "full" not in _NC_CACHE:
        _NC_CACHE["full"] = _build_nc()
    return _NC_CACHE["full"]


def _pack_tables(emb_table, conv_w, conv_b, lin_w, lin_b):
    """Host-side table precompute + fp8/fp16 packing (a weight repack)."""
    emb_table = np.asarray(emb_table, np.float32)
    conv_w = np.asarray(conv_w, np.float32)
    lin_w = np.asarray(lin_w, np.float32)
    # U[v,k,h] = sum_e emb[v,e] * conv_w[h,e,k]
    U = (emb_table @ conv_w.transpose(1, 0, 2).reshape(E, H * K))
    U = U.reshape(V, H, K).transpose(0, 2, 1).copy()       # [V, K, H]
    U[:, K - 1, :] += np.asarray(conv_b, np.float32)
    P1 = emb_table @ lin_w[:, :E].T + np.asarray(lin_b, np.float32)[None, :]
    W2T = lin_w[:, E:].T.copy()                            # [H, V]

    # u8[p, g, k, vh, i, m] = U[vh*128+p, k, (g*4+i)*128+m]
    u8 = (U.reshape(2, 128, K, 2, 4, 128)       # [vh, p, k, g, i, m]
          .transpose(1, 3, 2, 0, 4, 5)).astype(F8)
    w2_p = W2T.reshape(H8, 128, V).transpose(1, 0, 2)      # [128, H8, V]
    return np.ascontiguousarray(u8), np.ascontiguousarray(w2_p.astype(F16)), P1


def _onehot(tokens):
    """[128, 2, B, SPAD] fp8, left-padded with 15 zero columns per sequence."""
    tok = np.asarray(tokens).astype(np.int64)
    oh = np.zeros((128, 2, B, SPAD), F8)
    t = tok.ravel()
    b_idx = np.repeat(np.arange(B), S)
    col = np.tile(np.arange(S), B) + PAD
    oh[t % 128, t // 128, b_idx, col] = 1
    return oh


def kernel(input_sequence, emb_table, conv_w, conv_b, lin_w, lin_b):
    global LAST_RESULT
    import os
    if not TRACE:
        # the container's antenv lacks the axon NTFF hook; make sure an
        # ambient BASS_TRACE can't route us into that import path
        os.environ["BASS_NEVER_TRACE"] = "1"
    else:
        os.environ.pop("BASS_NEVER_TRACE", None)
    from concourse.bass_utils import run_bass_kernel_spmd

    u8, w2_p, P1 = _pack_tables(emb_table, conv_w, conv_b, lin_w, lin_b)
    oh_full = _onehot(input_sequence)
    # emb-side logits: gather P1 rows per token, packed per 512-token tile
    # as [tile, p, m, v] with token row = tile*512 + m*128 + p
    tok = np.asarray(input_sequence).astype(np.int64)
    pe_all = P1[tok].astype(np.float32)                      # [B, S, V]
    pe_all = (pe_all.reshape(B * S // 512, 4, 128, V)
              .transpose(0, 2, 1, 3))                  # [tiles, 128, 4, V]

    ntt_core = SEQ_PER_CORE * NTT
    in_maps = []
    for c in range(NCORES):
        in_maps.append({
            "oh": np.ascontiguousarray(
                oh_full[:, :, c * SEQ_PER_CORE:(c + 1) * SEQ_PER_CORE, :]),
            "u": u8, "w2": w2_p,
            "pe": np.ascontiguousarray(
                pe_all[c * ntt_core:(c + 1) * ntt_core]),
        })

    nc = _get_nc()
    res = run_bass_kernel_spmd(nc, in_maps, core_ids=list(range(NCORES)),
                               trace=TRACE)
    LAST_RESULT = res
    outs = [res.results[c]["out"] for c in range(NCORES)]   # [8192, 256] each
    full = np.concatenate(outs, axis=0).reshape(B, S, V)
    return np.ascontiguousarray(full.astype(np.float32))


# revision 11
# speedup vs baseline: 1.0009x; 1.0009x over previous
"""Trainium2 Bass kernel for CharPredictorMultirateFFN.

Model: emb = emb_table[tokens]; conv = relu(causal_conv1d(emb, K=16) + b);
logits = cat(emb, conv) @ lin_w.T + lin_b; out = softmax(logits).

Key algebraic restructure (tokens take only V=256 values):
  conv[s, h] = sum_k U[tok[s-15+k], k, h]   with U[v,k,h] = sum_e emb[v,e] conv_w[h,e,k]
so the conv becomes 16 shifted one-hot matmuls with contract dim 256 (half the
FLOPs of the direct E=512 conv) and the one-hot operand is exact in fp8.
The emb half of the final linear folds into P1 = emb_table @ lin_w[:, :E].T
(one-hot matmul, [256,256]), removing the embedding gather entirely.

fp8 DoubleRow: the PE's MatmulPerfMode.DoubleRow packs the 2 v-halves of the
256-wide one-hot contraction into ONE matmul (lhsT [128,2,128], rhs [128,2,512])
running at 2x the fp16 MAC rate. U is quantized to e4m3 host-side (the one-hot
is exact in fp8); measured end-to-end rel_l2 ~1.5e-2 vs the 2e-2 gate.

Sharding: data-parallel over batch — 4 sequences per core on 8 cores, all
tables replicated, no collectives.

biases are folded host-side: conv_b into U[:, K-1, :] (tap k=15 is always
valid for every output position), lin_b into P1 rows (shift-0 one-hot always
valid), so the device kernel has no bias adds.
"""

import numpy as np
import ml_dtypes

B, S, V, E, H, K = 32, 2048, 256, 512, 1024, 16
NCORES = 8
SEQ_PER_CORE = B // NCORES            # 4
PAD = K - 1                           # 15
SPAD = S + PAD                        # 2063
H8 = H // 128                         # 8
NTT = S // 512                        # 4 token-tiles of 512 per sequence
F16 = np.float16
F8 = ml_dtypes.float8_e4m3

TRACE = False          # set True (e.g. from test.py) to capture NTFF profile
LAST_RESULT = None     # BassKernelResults of the most recent run

_NC_CACHE = {}


def _build_nc(seq_per_core=SEQ_PER_CORE, ntt=NTT):
    """Build the Bass module (SPMD, identical program on every core)."""
    from contextlib import ExitStack
    import concourse.bacc as bacc
    import concourse.tile as tile
    import concourse.mybir as mybir

    f32 = mybir.dt.float32
    f16 = mybir.dt.float16
    f8 = mybir.dt.float8e4
    AF = mybir.ActivationFunctionType
    DR = mybir.MatmulPerfMode.DoubleRow
    toks = seq_per_core * ntt * 512

    nc = bacc.Bacc("TRN2", target_bir_lowering=False, debug=False,
                   num_devices=NCORES)

    # u layout: [part, g(h-half), k, vh, i(h-chunk), 128] so each conv group's
    # weights are contiguous per-k slabs that can land early via sliced DMAs.
    # The lhsT AP for DoubleRow is u[:, g, k, :, i, :] = [128, 2(vh), 128].
    oh_d = nc.dram_tensor("oh", [128, 2, seq_per_core, SPAD], f8,
                          kind="ExternalInput").ap()
    u_d = nc.dram_tensor("u", [128, 2, K, 2, 4, 128], f8,
                         kind="ExternalInput").ap()
    w2_d = nc.dram_tensor("w2", [128, H8, V], f16,
                          kind="ExternalInput").ap()
    # host-gathered P1[tok] rows (emb half of the linear; lin_b folded in):
    # [tile, p, m, v] = row tile*512 + m*128 + p, so each token-tile is one
    # contiguous [128, 4, V] DMA.
    pe_d = nc.dram_tensor("pe", [seq_per_core * ntt, 128, 4, V], f16,
                          kind="ExternalInput").ap()
    out_d = nc.dram_tensor("out", [toks, V], f32, kind="ExternalOutput").ap()

    with tile.TileContext(nc) as tc, ExitStack() as ctx:
        consts = ctx.enter_context(tc.tile_pool(name="consts", bufs=1))
        u_t = consts.tile([128, 2, K, 2, 4, 128], f8, name="u_t")
        oh_t = consts.tile([128, 2, seq_per_core, SPAD], f8, name="oh_t")
        w2_t = consts.tile([128, H8, V], f16, name="w2_t")
        # staggered loads ordered along the kernel's critical path: the
        # first conv group consumes u[:, 0, k] in k order on oh[b=0,
        # cols<528], so stream those slabs first in small chunks. The oh
        # head chunk goes out on the Activation HWDGE queue so its issue
        # doesn't serialize behind the u chunks on SP.
        nc.scalar.dma_start(oh_t[:, :, 0, 0:528], oh_d[:, :, 0, 0:528])
        for kq in range(K):
            eng = nc.sync if kq % 2 == 0 else nc.scalar
            eng.dma_start(u_t[:, 0, kq:kq + 1], u_d[:, 0, kq:kq + 1])
        KQ = 2
        for kq in range(0, K, KQ):
            nc.sync.dma_start(u_t[:, 1, kq:kq + KQ],
                              u_d[:, 1, kq:kq + KQ])
        nc.sync.dma_start(oh_t[:, :, 0, 528:SPAD], oh_d[:, :, 0, 528:SPAD])
        for b in range(1, seq_per_core):
            nc.sync.dma_start(oh_t[:, :, b, :], oh_d[:, :, b, :])
        nc.sync.dma_start(w2_t[:], w2_d[:])

        pe_pool = ctx.enter_context(tc.tile_pool(name="pep", bufs=3))
        r_pool = ctx.enter_context(tc.tile_pool(name="rp", bufs=3))
        cps = ctx.enter_context(tc.tile_pool(name="cps", bufs=6, space="PSUM"))

        # PE warm-up: the HAM clock gate holds the PE at 1.2 GHz until it
        # has been busy ~3.4us. Run throwaway matmuls while the input DMAs
        # are in flight so the real stream starts at 2.4 GHz with no cold
        # ramp. Operands are deliberately uninitialized (no memset: that
        # would wait on the Vector engine's preamble and delay the PE);
        # the results land in a PSUM bank that the first real accumulation
        # group resets via start=True.
        wlhs = consts.tile([128, 128], f16, name="wlhs")
        wrhs = consts.tile([128, 512], f16, name="wrhs")
        nc.gpsimd.memset(wlhs[:], 0)
        nc.gpsimd.memset(wrhs[:], 0)
        wp = cps.tile([128, 512], f32, name="warmps", tag="cp")
        for _ in range(10):
            nc.tensor.matmul(wp[:], wlhs[:], wrhs[:], start=True, stop=True)
        lps = ctx.enter_context(tc.tile_pool(name="lps", bufs=2, space="PSUM"))
        sm_pool = ctx.enter_context(tc.tile_pool(name="smp", bufs=8))
        out_pool = ctx.enter_context(tc.tile_pool(name="outp", bufs=6))

        def conv_group(b, tt, chunks, rt):
            """Conv for the given h-chunks of 512 tokens -> relu -> rt.

            Groups of <=3 chunks with 6 PSUM bufs mean consecutive groups
            use disjoint banks, so each group's relu has a full group
            (>=32 matmuls, ~7us) before its banks are reused: no PE stall
            waiting on ACT."""
            col0 = tt * 512
            ps = [cps.tile([128, 512], f32, name=f"cp{i}", tag="cp")
                  for i in range(len(chunks))]
            for k in range(K):
                rhs = oh_t[:, :, b, col0 + k: col0 + k + 512]
                for j, c in enumerate(chunks):
                    nc.tensor.matmul(
                        ps[j][:],
                        u_t[:, c // 4, k, :, c % 4, :],
                        rhs,
                        start=(k == 0), stop=(k == K - 1),
                        perf_mode=DR)
            for j, c in enumerate(chunks):
                nc.scalar.activation(rt[:, c, :], ps[j][:], AF.Relu)

        def stage3_m(b, tt, rt, pe_t, m):
            """One 128-token block: logits = R@W2T + P1 rows, softmax, DMA."""
            psl = lps.tile([128, V], f32, name="psl", tag="psl")
            for h8 in range(H8):
                nc.tensor.matmul(
                    psl[:], rt[:, h8, m * 128:(m + 1) * 128],
                    w2_t[:, h8, :],
                    start=(h8 == 0), stop=(h8 == H8 - 1))
            li = sm_pool.tile([128, V], f32, name="li", tag="li")
            nc.vector.tensor_add(li[:], psl[:], pe_t[:, m, :])
            et = sm_pool.tile([128, V], f32, name="et", tag="et")
            ssum = sm_pool.tile([128, 1], f32, name="ssum", tag="ssum")
            nc.scalar.activation(et[:], li[:], AF.Exp, accum_out=ssum[:])
            rec = sm_pool.tile([128, 1], f32, name="rec", tag="rec")
            nc.vector.reciprocal(rec[:], ssum[:])
            ot = out_pool.tile([128, V], f32, name="ot", tag="ot")
            nc.vector.tensor_scalar_mul(ot[:], et[:], rec[:])
            row0 = (b * ntt + tt) * 512 + m * 128
            nc.sync.dma_start(out_d[row0:row0 + 128, :], ot[:])

        # software pipeline at conv-group granularity: halves of tile i-1's
        # stage3 run on the PE between tile i's conv groups, spreading the
        # ACT/DVE softmax load and halving the final tile's PE-idle tail.
        tiles = [(b, tt) for b in range(seq_per_core) for tt in range(ntt)]
        prev = None
        for (b, tt) in tiles:
            pe_t = pe_pool.tile([128, 4, V], f16, name="pe_t", tag="pe")
            nc.sync.dma_start(pe_t[:], pe_d[b * ntt + tt])
            rt = r_pool.tile([128, H8, 512], f16, name="rt", tag="rt")
            conv_group(b, tt, (0, 1, 2), rt)
            if prev is not None:
                stage3_m(*prev, 0)
                stage3_m(*prev, 1)
            conv_group(b, tt, (3, 4, 5), rt)
            if prev is not None:
                stage3_m(*prev, 2)
                stage3_m(*prev, 3)
            conv_group(b, tt, (6, 7), rt)
            prev = (b, tt, rt, pe_t)
        for m in range(4):
            stage3_m(*prev, m)

    nc.compile()
    return nc


def _get_nc():
    if "full" not in _NC_CACHE:
        _NC_CACHE["full"] = _build_nc()
    return _NC_CACHE["full"]


def _pack_tables(emb_table, conv_w, conv_b, lin_w, lin_b):
    """Host-side table precompute + fp8/fp16 packing (a weight repack)."""
    emb_table = np.asarray(emb_table, np.float32)
    conv_w = np.asarray(conv_w, np.float32)
    lin_w = np.asarray(lin_w, np.float32)
    # U[v,k,h] = sum_e emb[v,e] * conv_w[h,e,k]
    U = (emb_table @ conv_w.transpose(1, 0, 2).reshape(E, H * K))
    U = U.reshape(V, H, K).transpose(0, 2, 1).copy()       # [V, K, H]
    U[:, K - 1, :] += np.asarray(conv_b, np.float32)
    P1 = emb_table @ lin_w[:, :E].T + np.asarray(lin_b, np.float32)[None, :]
    W2T = lin_w[:, E:].T.copy()                            # [H, V]

    # u8[p, g, k, vh, i, m] = U[vh*128+p, k, (g*4+i)*128+m]
    u8 = (U.reshape(2, 128, K, 2, 4, 128)       # [vh, p, k, g, i, m]
          .transpose(1, 3, 2, 0, 4, 5)).astype(F8)
    w2_p = W2T.reshape(H8, 128, V).transpose(1, 0, 2)      # [128, H8, V]
    return np.ascontiguousarray(u8), np.ascontiguousarray(w2_p.astype(F16)), P1


def _onehot(tokens):
    """[128, 2, B, SPAD] fp8, left-padded with 15 zero columns per sequence."""
    tok = np.asarray(tokens).astype(np.int64)
    oh = np.zeros((128, 2, B, SPAD), F8)
    t = tok.ravel()
    b_idx = np.repeat(np.arange(B), S)
    col = np.tile(np.arange(S), B) + PAD
    oh[t % 128, t // 128, b_idx, col] = 1
    return oh


def kernel(input_sequence, emb_table, conv_w, conv_b, lin_w, lin_b):
    global LAST_RESULT
    import os
    if not TRACE:
        # the container's antenv lacks the axon NTFF hook; make sure an
        # ambient BASS_TRACE can't route us into that import path
        os.environ["BASS_NEVER_TRACE"] = "1"
    else:
        os.environ.pop("BASS_NEVER_TRACE", None)
    from concourse.bass_utils import run_bass_kernel_spmd

    u8, w2_p, P1 = _pack_tables(emb_table, conv_w, conv_b, lin_w, lin_b)
    oh_full = _onehot(input_sequence)
    # emb-side logits: gather P1 rows per token, packed per 512-token tile
    # as [tile, p, m, v] with token row = tile*512 + m*128 + p
    tok = np.asarray(input_sequence).astype(np.int64)
    pe_all = P1[tok].astype(np.float16)                      # [B, S, V]
    pe_all = (pe_all.reshape(B * S // 512, 4, 128, V)
              .transpose(0, 2, 1, 3))                  # [tiles, 128, 4, V]

    ntt_core = SEQ_PER_CORE * NTT
    in_maps = []
    for c in range(NCORES):
        in_maps.append({
            "oh": np.ascontiguousarray(
                oh_full[:, :, c * SEQ_PER_CORE:(c + 1) * SEQ_PER_CORE, :]),
            "u": u8, "w2": w2_p,
            "pe": np.ascontiguousarray(
                pe_all[c * ntt_core:(c + 1) * ntt_core]),
        })

    nc = _get_nc()
    res = run_bass_kernel_spmd(nc, in_maps, core_ids=list(range(NCORES)),
                               trace=TRACE)
    LAST_RESULT = res
    outs = [res.results[c]["out"] for c in range(NCORES)]   # [8192, 256] each
    full = np.concatenate(outs, axis=0).reshape(B, S, V)
    return np.ascontiguousarray(full.astype(np.float32))
